# revision 3
# baseline (speedup 1.0000x reference)
"""Trainium2 Bass kernel for nn_CompressModel (Golay-modulated FWHT + global RMS).

Math:
  x_flat = x.reshape(-1, 1024); x_mod = x_flat * golay
  y = FWHT_1024(x_mod);  alpha = sqrt(mean(y**2) + 1e-5)  (global over all elems)
  out = y / (alpha + 1e-5)

Key identities used:
  * FWHT_1024 = H128 (over low 7 feature bits) (x) H8 (over top 3 bits).
  * Parseval: sum(y**2) == 1024 * sum(x_mod**2) == 1024 * sum(x**2)  (golay is +-1),
    so alpha is computed from the INPUT, needing no second pass over y.

Per-core layout (data-parallel over rows, 4096 rows/core):
  Host supplies xT = x_shard.T  (feature-major [1024, 4096]) so that the
  contraction dim of every matmul is the SBUF partition dim — no on-device
  transposes.  Stationary matrices D_h = diag(golay_chunk_h) @ H128 (and
  negated copies) fold the Golay modulation into the FWHT weights.
  TensorE folds the top FOLD bits of the H8 factor via PSUM accumulation;
  VectorE butterflies the remaining 3-FOLD bits; the global RMS is an
  AllReduce of a [128,1] partial-sum vector; final scaling is a per-partition
  tensor_scalar multiply fused in before the output DMA.
"""

import os
import sys

import numpy as np

try:
    import concourse.bass as bass  # noqa: F401
except ImportError:  # pragma: no cover
    for _p in ("/opt/trn_rl_repo", "/root/.axon_site/_ro/trn_rl_repo"):
        if os.path.isdir(_p) and _p not in sys.path:
            sys.path.insert(0, _p)
    import concourse.bass as bass  # noqa: F401

import concourse.bacc as bacc
import concourse.mybir as mybir
import concourse.tile as tile
from concourse import bass_utils

N_CORES = 8
DIM = 1024
P = 128
NH = DIM // P            # 8 feature chunks
ROWS = 8 * 4096          # total rows
RC = ROWS // N_CORES     # rows per core = 4096
NBLK_N = 512             # rows per block (one PSUM bank of fp32)
NBLK = RC // NBLK_N      # 8 blocks per core
EPS = 1e-5
MEAN_SCALE = float(DIM) / float(ROWS * DIM)  # sum(x^2)*this == mean(y^2)

FOLD = 2                 # top h-bits folded into TensorE PSUM accumulation
B_REM = 3 - FOLD         # h-bits handled by VectorE butterflies
MM_DT = mybir.dt.float32r


def _h128() -> np.ndarray:
    a = np.arange(P)
    pc = np.zeros((P, P), dtype=np.int64)
    m = a[:, None] & a[None, :]
    for bit in range(7):
        pc ^= (m >> bit) & 1
    return np.where(pc == 0, 1.0, -1.0).astype(np.float32)


def _build_dmats(golay: np.ndarray) -> np.ndarray:
    """[2*NH, 128, 128]: slot h = diag(g_h) @ H128, slot NH+h = negated."""
    H = _h128()
    d = np.empty((2 * NH, P, P), dtype=np.float32)
    for h in range(NH):
        d[h] = golay[P * h : P * h + P, None] * H
        d[NH + h] = -d[h]
    # SBUF layout: partition l, free = i*128 + c  (one contiguous DMA)
    return np.ascontiguousarray(d.transpose(1, 0, 2).reshape(P, 2 * NH * P))


def _build_nc():
    nc = bacc.Bacc(
        "TRN2", target_bir_lowering=False, debug=False, num_devices=N_CORES
    )
    xT = nc.dram_tensor("xT", [DIM, RC], MM_DT, kind="ExternalInput").ap()
    dm = nc.dram_tensor("dmats", [P, 2 * NH * P], MM_DT, kind="ExternalInput").ap()
    yT = nc.dram_tensor("yT", [DIM, RC], mybir.dt.float32, kind="ExternalOutput").ap()

    f32 = mybir.dt.float32
    NF = 2 ** FOLD       # folded input chunks per accumulation group
    NU = 2 ** B_REM      # unfolded chunks (vector butterfly width)

    with tile.TileContext(nc) as tc:
        with (
            tc.tile_pool(name="dpool", bufs=1) as dpool,
            tc.tile_pool(name="xin", bufs=10) as xin_pool,
            tc.tile_pool(name="ysb", bufs=NH * NBLK) as ysb_pool,
            tc.tile_pool(name="gsb", bufs=6) as gsb_pool,
            tc.tile_pool(name="sq", bufs=2) as sq_pool,
            tc.tile_pool(name="small", bufs=1) as small,
            tc.tile_pool(name="ps", bufs=NH, space="PSUM") as ps_pool,
            tc.tile_pool(name="dram", bufs=1, space="DRAM") as dr_pool,
        ):
            # stationary matrices, resident
            d_sb = dpool.tile([P, 2 * NH * P], MM_DT)
            nc.sync.dma_start(d_sb[:], dm[:])

            XW = 2 * NBLK_N        # input tile width (2 row blocks)
            partials = small.tile([P, NH * (NBLK // 2)], f32)
            y_tiles = {}
            xtiles = {}

            for jj in range(NBLK // 2):
                # ---- load the 8 feature-chunk tiles of this block pair ----
                for h in range(NH):
                    x_t = xin_pool.tile([P, XW], MM_DT, name="x_t", tag="x_t")
                    nc.sync.dma_start(
                        x_t[:],
                        xT[P * h : P * h + P, XW * jj : XW * (jj + 1)],
                    )
                    xtiles[(jj, h)] = x_t
                    # sumsq partial on ScalarE (reads raw fp32 bytes)
                    sq_t = sq_pool.tile([P, XW], f32, name="sq_scratch")
                    nc.scalar.activation(
                        sq_t[:],
                        x_t.bitcast(f32)[:],
                        mybir.ActivationFunctionType.Square,
                        accum_out=partials[:, jj * NH + h : jj * NH + h + 1],
                    )

            for j in range(NBLK):
                jj, half = j // 2, j % 2
                xt = {
                    h: xtiles[(jj, h)][:, NBLK_N * half : NBLK_N * (half + 1)]
                    for h in range(NH)
                }
                # ---- TensorE: H128 + folded top bits of H8 ----
                G = {}
                for fo in range(NF):
                    for u in range(NU):
                        ps_t = ps_pool.tile([P, NBLK_N], f32, name="psum_g")
                        for t, fi in enumerate(range(NF)):
                            sign_neg = (bin(fo & fi).count("1") & 1) == 1
                            chunk = fi * NU + u
                            slot = chunk + (NH if sign_neg else 0)
                            nc.tensor.matmul(
                                ps_t[:],
                                d_sb[:, P * slot : P * slot + P],
                                xt[chunk],
                                start=(t == 0),
                                stop=(t == NF - 1),
                            )
                        G[(fo, u)] = ps_t

                # ---- VectorE butterflies over remaining bits ----
                for fo in range(NF):
                    tiles = [G[(fo, u)] for u in range(NU)]
                    in_psum = True
                    for s in range(B_REM):
                        d = NU >> (s + 1)
                        nxt = [None] * NU
                        for base in range(0, NU, 2 * d):
                            for i in range(base, base + d):
                                a, b = tiles[i], tiles[i + d]
                                if in_psum:
                                    # copy one operand out of PSUM (TT cannot
                                    # read two PSUM inputs)
                                    c = gsb_pool.tile(
                                        [P, NBLK_N], f32, name="g_copy"
                                    )
                                    nc.vector.tensor_copy(c[:], b[:])
                                    b = c
                                if s == B_REM - 1:
                                    op = ysb_pool
                                else:
                                    op = gsb_pool
                                tp = op.tile([P, NBLK_N], f32, name=f"bf_{s}_p", tag=f"bf_{s}_p", bufs=(NF * NU // 2) * NBLK if s == B_REM - 1 else None)
                                tm = op.tile([P, NBLK_N], f32, name=f"bf_{s}_m", tag=f"bf_{s}_m", bufs=(NF * NU // 2) * NBLK if s == B_REM - 1 else None)
                                nc.vector.tensor_tensor(
                                    tp[:], a[:], b[:], mybir.AluOpType.add
                                )
                                nc.vector.tensor_tensor(
                                    tm[:], a[:], b[:], mybir.AluOpType.subtract
                                )
                                nxt[i] = tp
                                nxt[i + d] = tm
                        tiles = nxt
                        in_psum = False
                    for uo in range(NU):
                        hp = fo * NU + uo
                        y_tiles[(j, hp)] = tiles[uo]

            # ---- global RMS: reduce partials, AllReduce, s = 1/(alpha+eps) ----
            part_sum = small.tile([P, 1], f32)
            nc.vector.tensor_reduce(
                part_sum[:], partials[:], axis=mybir.AxisListType.X,
                op=mybir.AluOpType.add,
            )
            ones_sb = small.tile([P, P], f32)
            nc.vector.memset(ones_sb[:], 1.0)
            tot_ps = ps_pool.tile([P, 1], f32, name="tot_ps", tag="psum_g")
            nc.tensor.matmul(
                tot_ps[:], ones_sb[:], part_sum[:], start=True, stop=True
            )
            tot_sb = small.tile([P, 1], f32)
            nc.scalar.copy(tot_sb[:], tot_ps[:])

            cc_in = dr_pool.tile([P, 1], f32)
            cc_out = dr_pool.tile([P, 1], f32, addr_space="Shared")
            nc.gpsimd.dma_start(cc_in[:], tot_sb[:])
            nc.gpsimd.collective_compute(
                "AllReduce",
                mybir.AluOpType.add,
                replica_groups=[list(range(N_CORES))],
                ins=[cc_in.opt()],
                outs=[cc_out.opt()],
            )
            g_sb = small.tile([P, 1], f32)
            nc.gpsimd.dma_start(g_sb[:], cc_out[:])

            eps_sb = small.tile([P, 1], f32)
            nc.vector.memset(eps_sb[:], EPS)
            alpha_sb = small.tile([P, 1], f32)
            nc.scalar.activation(
                alpha_sb[:], g_sb[:], mybir.ActivationFunctionType.Sqrt,
                bias=eps_sb[:], scale=MEAN_SCALE,
            )
            alpha_eps = small.tile([P, 1], f32)
            nc.scalar.activation(
                alpha_eps[:], alpha_sb[:], mybir.ActivationFunctionType.Identity,
                bias=eps_sb[:], scale=1.0,
            )
            s_pp = small.tile([P, 1], f32)
            nc.vector.reciprocal(s_pp[:], alpha_eps[:])

            # ---- scale in place and store ----
            for j in range(NBLK):
                for hp in range(NH):
                    y_t = y_tiles[(j, hp)]
                    nc.vector.tensor_scalar_mul(y_t[:], y_t[:], s_pp[:])
                    nc.sync.dma_start(
                        yT[P * hp : P * hp + P, NBLK_N * j : NBLK_N * (j + 1)],
                        y_t[:],
                    )
    nc.compile()
    return nc


_NC_CACHE = None


def _get_nc():
    global _NC_CACHE
    if _NC_CACHE is None:
        _NC_CACHE = _build_nc()
    return _NC_CACHE


def run(inputs: dict, trace: bool = False):
    x = np.asarray(inputs["x"], dtype=np.float32)
    golay = np.asarray(inputs["golay"], dtype=np.float32).reshape(DIM)
    orig_shape = x.shape
    x2 = np.ascontiguousarray(x).reshape(ROWS, DIM)

    dmats = _build_dmats(golay)
    in_maps = []
    for c in range(N_CORES):
        shard = np.ascontiguousarray(x2[c * RC : (c + 1) * RC, :].T)
        in_maps.append({"xT": shard, "dmats": dmats})

    nc = _get_nc()
    res = bass_utils.run_bass_kernel_spmd(
        nc, in_maps, core_ids=list(range(N_CORES)), trace=trace
    )

    y2 = np.empty((ROWS, DIM), dtype=np.float32)
    for c in range(N_CORES):
        y2[c * RC : (c + 1) * RC, :] = res.results[c]["yT"].T
    return y2.reshape(orig_shape), res


def kernel(**inputs) -> np.ndarray:
    out, _ = run(inputs, trace=False)
    return out


# revision 6
# speedup vs baseline: 1.2916x; 1.2916x over previous
"""Trainium2 Bass kernel for nn_CompressModel (Golay-modulated FWHT + global RMS).

Math:
  x_flat = x.reshape(-1, 1024); x_mod = x_flat * golay
  y = FWHT_1024(x_mod);  alpha = sqrt(mean(y**2) + 1e-5)  (global over all elems)
  out = y / (alpha + 1e-5)

Key identities used:
  * FWHT_1024 = H128 (over low 7 feature bits) (x) H8 (over top 3 bits).
  * Parseval: sum(y**2) == 1024 * sum(x_mod**2) == 1024 * sum(x**2)  (golay is +-1),
    so alpha is computed from the INPUT, needing no second pass over y.

Per-core layout (data-parallel over rows, 4096 rows/core):
  Host supplies xT = x_shard.T  (feature-major [1024, 4096]) so that the
  contraction dim of every matmul is the SBUF partition dim — no on-device
  transposes.  Stationary matrices D_h = diag(golay_chunk_h) @ H128 (and
  negated copies) fold the Golay modulation into the FWHT weights.
  TensorE folds the top FOLD bits of the H8 factor via PSUM accumulation;
  VectorE butterflies the remaining 3-FOLD bits; the global RMS is an
  AllReduce of a [128,1] partial-sum vector; final scaling is a per-partition
  tensor_scalar multiply fused in before the output DMA.
"""

import os
import sys

import numpy as np

try:
    import concourse.bass as bass  # noqa: F401
except ImportError:  # pragma: no cover
    for _p in ("/opt/trn_rl_repo", "/root/.axon_site/_ro/trn_rl_repo"):
        if os.path.isdir(_p) and _p not in sys.path:
            sys.path.insert(0, _p)
    import concourse.bass as bass  # noqa: F401

import concourse.bacc as bacc
import concourse.mybir as mybir
import concourse.tile as tile
from concourse import bass_utils

N_CORES = 8
DIM = 1024
P = 128
NH = DIM // P            # 8 feature chunks
ROWS = 8 * 4096          # total rows
RC = ROWS // N_CORES     # rows per core = 4096
NBLK_N = 512             # rows per block (one PSUM bank of fp32)
NBLK = RC // NBLK_N      # 8 blocks per core
EPS = 1e-5
MEAN_SCALE = float(DIM) / float(ROWS * DIM)  # sum(x^2)*this == mean(y^2)

FOLD = 2                 # top h-bits folded into TensorE PSUM accumulation
B_REM = 3 - FOLD         # h-bits handled by VectorE butterflies
MM_DT = mybir.dt.float32r


def _h128() -> np.ndarray:
    a = np.arange(P)
    pc = np.zeros((P, P), dtype=np.int64)
    m = a[:, None] & a[None, :]
    for bit in range(7):
        pc ^= (m >> bit) & 1
    return np.where(pc == 0, 1.0, -1.0).astype(np.float32)


def _build_dmats(golay: np.ndarray) -> np.ndarray:
    """[2*NH, 128, 128]: slot h = diag(g_h) @ H128, slot NH+h = negated."""
    H = _h128()
    d = np.empty((2 * NH, P, P), dtype=np.float32)
    for h in range(NH):
        d[h] = golay[P * h : P * h + P, None] * H
        d[NH + h] = -d[h]
    # SBUF layout: partition l, free = i*128 + c  (one contiguous DMA)
    return np.ascontiguousarray(d.transpose(1, 0, 2).reshape(P, 2 * NH * P))


def _build_nc():
    nc = bacc.Bacc(
        "TRN2", target_bir_lowering=False, debug=False, num_devices=N_CORES
    )
    xT = nc.dram_tensor("xT", [DIM, RC], MM_DT, kind="ExternalInput").ap()
    dm = nc.dram_tensor("dmats", [P, 2 * NH * P], MM_DT, kind="ExternalInput").ap()
    yT = nc.dram_tensor("yT", [DIM, RC], mybir.dt.float32, kind="ExternalOutput").ap()

    f32 = mybir.dt.float32
    NF = 2 ** FOLD       # folded input chunks per accumulation group
    NU = 2 ** B_REM      # unfolded chunks (vector butterfly width)
    XW = 2 * NBLK_N      # input tile width (2 row blocks)
    NJJ = NBLK // 2
    FUSE_FROM = NBLK // 2  # blocks >= this use scale-fused butterflies

    with tile.TileContext(nc) as tc:
        with (
            tc.tile_pool(name="dpool", bufs=1) as dpool,
            tc.tile_pool(name="xin", bufs=12) as xin_pool,
            tc.tile_pool(name="ysb", bufs=44) as ysb_pool,
            tc.tile_pool(name="gsb", bufs=4) as gsb_pool,
            tc.tile_pool(name="sq", bufs=2) as sq_pool,
            tc.tile_pool(name="small", bufs=1) as small,
            tc.tile_pool(name="ps", bufs=NH, space="PSUM") as ps_pool,
            tc.tile_pool(name="dram", bufs=1, space="DRAM") as dr_pool,
        ):
            # stationary matrices, resident (one contiguous DMA)
            d_sb = dpool.tile([P, 2 * NH * P], MM_DT)
            nc.sync.dma_start(d_sb[:], dm[:])

            partials = small.tile([P, NH * NJJ], f32)
            xtiles = {}

            # ---- phase 1: input DMA + sumsq partials (ScalarE) ----
            for jj in range(NJJ):
                for h in range(NH):
                    if jj < NJJ // 2:
                        x_t = xin_pool.tile([P, XW], MM_DT, name="x_t",
                                            tag="x_early", bufs=8)
                    else:
                        # dedicated slots: these DMAs must never wait on a
                        # release (the s_pp chain depends on them)
                        x_t = xin_pool.tile([P, XW], MM_DT, name="x_t",
                                            tag="x_late", bufs=NH * (NJJ - NJJ // 2))
                    nc.sync.dma_start(
                        x_t[:], xT[P * h : P * h + P, XW * jj : XW * (jj + 1)]
                    )
                    xtiles[(jj, h)] = x_t
                    sq_t = sq_pool.tile([P, XW], f32, name="sq_scratch")
                    nc.scalar.activation(
                        sq_t[:],
                        x_t.bitcast(f32)[:],
                        mybir.ActivationFunctionType.Square,
                        accum_out=partials[:, jj * NH + h : jj * NH + h + 1],
                    )

            # ---- alpha chain (emitted early for priority; gated by partials) ----
            part_sum = small.tile([P, 1], f32)
            nc.vector.tensor_reduce(
                part_sum[:], partials[:], axis=mybir.AxisListType.X,
                op=mybir.AluOpType.add,
            )
            ones_sb = small.tile([P, P], f32)
            nc.vector.memset(ones_sb[:], 1.0)
            tot_ps = ps_pool.tile([P, 1], f32, name="tot_ps", tag="tot_ps", bufs=1)
            nc.tensor.matmul(tot_ps[:], ones_sb[:], part_sum[:], start=True, stop=True)
            tot_sb = small.tile([P, 1], f32)
            nc.scalar.copy(tot_sb[:], tot_ps[:])

            cc_in = dr_pool.tile([P, 1], f32)
            cc_out = dr_pool.tile([N_CORES * P, 1], f32, addr_space="Shared")
            nc.gpsimd.dma_start(cc_in[:], tot_sb[:])
            nc.gpsimd.collective_compute(
                "AllGather",
                mybir.AluOpType.bypass,
                replica_groups=[list(range(N_CORES))],
                ins=[cc_in.opt()],
                outs=[cc_out.opt()],
            )
            gat_sb = small.tile([P, N_CORES], f32)
            nc.gpsimd.dma_start(
                gat_sb[:],
                cc_out.rearrange("(c p) o -> p (c o)", c=N_CORES),
            )
            g_sb = small.tile([P, 1], f32)
            nc.vector.tensor_reduce(
                g_sb[:], gat_sb[:], axis=mybir.AxisListType.X,
                op=mybir.AluOpType.add,
            )
            eps_sb = small.tile([P, 1], f32)
            nc.vector.memset(eps_sb[:], EPS)
            alpha_sb = small.tile([P, 1], f32)
            nc.scalar.activation(
                alpha_sb[:], g_sb[:], mybir.ActivationFunctionType.Sqrt,
                bias=eps_sb[:], scale=MEAN_SCALE,
            )
            alpha_eps = small.tile([P, 1], f32)
            nc.scalar.activation(
                alpha_eps[:], alpha_sb[:], mybir.ActivationFunctionType.Identity,
                bias=eps_sb[:], scale=1.0,
            )
            s_pp = small.tile([P, 1], f32)
            nc.vector.reciprocal(s_pp[:], alpha_eps[:])

            # ---- phase 2: per-block FWHT ----
            y_tiles = {}
            for j in range(NBLK):
                jj, half = j // 2, j % 2
                fused = j >= FUSE_FROM
                xt = {
                    h: xtiles[(jj, h)][:, NBLK_N * half : NBLK_N * (half + 1)]
                    for h in range(NH)
                }
                G = {}
                for fo in range(NF):
                    for u in range(NU):
                        ps_t = ps_pool.tile([P, NBLK_N], f32, name="psum_g", tag="psum_g", bufs=7)
                        for t, fi in enumerate(range(NF)):
                            sign_neg = (bin(fo & fi).count("1") & 1) == 1
                            chunk = fi * NU + u
                            slot = chunk + (NH if sign_neg else 0)
                            nc.tensor.matmul(
                                ps_t[:],
                                d_sb[:, P * slot : P * slot + P],
                                xt[chunk],
                                start=(t == 0),
                                stop=(t == NF - 1),
                            )
                        G[(fo, u)] = ps_t

                # butterfly over the remaining bit (FOLD=2: one stage)
                for fo in range(NF):
                    a, b = G[(fo, 0)], G[(fo, 1)]
                    tp = ysb_pool.tile([P, NBLK_N], f32, name="y_p", tag="y_p", bufs=18)
                    tm = ysb_pool.tile([P, NBLK_N], f32, name="y_m", tag="y_m", bufs=18)
                    if fused:
                        c = gsb_pool.tile([P, NBLK_N], f32, name="g_copy")
                        nc.scalar.activation(
                            c[:], b[:], mybir.ActivationFunctionType.Copy,
                            scale=s_pp[:],
                        )
                        nc.vector.scalar_tensor_tensor(
                            tp[:], a[:], s_pp[:], c[:],
                            mybir.AluOpType.mult, mybir.AluOpType.add,
                        )
                        nc.vector.scalar_tensor_tensor(
                            tm[:], a[:], s_pp[:], c[:],
                            mybir.AluOpType.mult, mybir.AluOpType.subtract,
                        )
                    else:
                        c = gsb_pool.tile([P, NBLK_N], f32, name="g_copy")
                        nc.scalar.copy(c[:], b[:])
                        nc.vector.tensor_tensor(
                            tp[:], a[:], c[:], mybir.AluOpType.add
                        )
                        nc.vector.tensor_tensor(
                            tm[:], a[:], c[:], mybir.AluOpType.subtract
                        )
                    for uo, y_t in ((0, tp), (1, tm)):
                        hp = fo * NU + uo
                        if fused:
                            nc.sync.dma_start(
                                yT[P * hp : P * hp + P,
                                   NBLK_N * j : NBLK_N * (j + 1)],
                                y_t[:],
                            )
                        else:
                            y_tiles[(j, hp)] = y_t

            # ---- phase 3: scale + store the pre-alpha blocks ----
            k = 0
            for j in range(FUSE_FROM):
                for hp in range(NH):
                    y_t = y_tiles[(j, hp)]
                    if k % 2 == 0:
                        nc.vector.tensor_scalar_mul(y_t[:], y_t[:], s_pp[:])
                    else:
                        nc.scalar.activation(
                            y_t[:], y_t[:], mybir.ActivationFunctionType.Copy,
                            scale=s_pp[:],
                        )
                    k += 1
                    nc.sync.dma_start(
                        yT[P * hp : P * hp + P, NBLK_N * j : NBLK_N * (j + 1)],
                        y_t[:],
                    )
    nc.compile()
    return nc


_NC_CACHE = None


def _get_nc():
    global _NC_CACHE
    if _NC_CACHE is None:
        _NC_CACHE = _build_nc()
    return _NC_CACHE


def run(inputs: dict, trace: bool = False):
    x = np.asarray(inputs["x"], dtype=np.float32)
    golay = np.asarray(inputs["golay"], dtype=np.float32).reshape(DIM)
    orig_shape = x.shape
    x2 = np.ascontiguousarray(x).reshape(ROWS, DIM)

    dmats = _build_dmats(golay)
    in_maps = []
    for c in range(N_CORES):
        shard = np.ascontiguousarray(x2[c * RC : (c + 1) * RC, :].T)
        in_maps.append({"xT": shard, "dmats": dmats})

    nc = _get_nc()
    res = bass_utils.run_bass_kernel_spmd(
        nc, in_maps, core_ids=list(range(N_CORES)), trace=trace
    )

    y2 = np.empty((ROWS, DIM), dtype=np.float32)
    for c in range(N_CORES):
        y2[c * RC : (c + 1) * RC, :] = res.results[c]["yT"].T
    return y2.reshape(orig_shape), res


def kernel(**inputs) -> np.ndarray:
    out, _ = run(inputs, trace=False)
    return out
